# revision 4
# baseline (speedup 1.0000x reference)
"""Trainium2 Bass kernel for nn_Circuit RK4 trajectory integration.

Math (mirrors the reference):
  A [B, 32] complex evolves under f(A) = i*(om + nu*|A|^2) .* A + A @ T2.T
  for 199 RK4 steps, emitting the state after every step.

Layout per core (B_local = 256 batch rows):
  - 2 independent streams of 128 batch rows each (pipeline fill).
  - Stream state Y: SBUF [128 part, 64 free] fp32 where
      partition p = c*64 + h*32 + m   (c = re/im, h = batch half, m = mode)
      free dim    b = batch row within half (64)
  - Complex linear op (Lc = T2 + i*diag(om), prescaled by the RK4 stage
    coefficient) is ONE [128,128] real block matmul on PE.
  - Nonlinearity: sq = Square(sqrt(s*nu)*Y) on ACT; a signed "sum the two
    squared halves" matmul on PE gives absw = (-+ s*nu*|y|^2 duplicated);
    th = absw .* swap(Y) on DVE; M' = M + I @ th accumulated on PE;
    Ynext = M' + A on DVE.
  - RK4 combine uses Y-identities:
      A' = (Y2 + 2*Y3 + Y4 - A)/3 + M4'   (M4' built with dt/6 scales)
"""

import os
import sys
import numpy as np

sys.path.insert(0, "/opt/trn_rl_repo")

MODES = 32
INPUT_MODES = 24
LAMBDA = 0.1
T_TOTAL = 1.0
EVAL_PTS = 200
NSTEPS = EVAL_PTS - 1
DT = T_TOTAL / (EVAL_PTS - 1)
NCORES = 8
BATCH = 2048
B_CORE = BATCH // NCORES  # 256
NSTREAM = 2
FD = B_CORE // NSTREAM // 2  # 64 batch per half


def _host_matrices(omega, kappa, nonlinearity, params):
    """Reproduce the reference's T2 computation (complex64, numpy)."""
    c64 = np.complex64
    n = MODES
    m = n * (n - 1) // 2
    re = params[:m].astype(np.float32)
    im = params[m : 2 * m].astype(np.float32)
    d = params[2 * m : 2 * m + n - 1].astype(np.float32)
    H = np.zeros((n, n), c64)
    iu = np.triu_indices(n, 1)
    H[iu] = re + 1j * im
    H = H + H.conj().T
    diag = np.concatenate([d, -np.sum(d, keepdims=True)]).astype(c64)
    H = H + np.diag(diag)
    w, V = np.linalg.eigh(H)
    U = ((V * np.exp(1j * w.astype(np.float32))[None, :]) @ V.conj().T).astype(c64)
    I = np.eye(n, dtype=c64)
    UtU = (U.T @ U).astype(c64)
    mix = UtU @ np.linalg.inv(I * (1.0 + LAMBDA) - UtU).astype(c64)
    kappa2 = kappa.astype(c64) ** 2
    sk = np.sqrt(kappa2)
    T2 = -(sk[:, None] * (0.5 * I + mix)) * sk[None, :]
    Lc = (T2 + 1j * np.diag(omega.astype(np.complex64))).astype(c64)
    nu = float(np.float32(nonlinearity[0]) ** 2)
    return Lc, nu


def _block_weights(Lc, scale):
    """Real [128,128] block matrix W so that W @ y applies scale*Lc per
    (c,h,m) layout p = c*64 + h*32 + m. Returns lhsT = W.T (f32)."""
    Lr = (scale * Lc.real).astype(np.float32)
    Li = (scale * Lc.imag).astype(np.float32)
    W = np.zeros((128, 128), np.float32)
    for h in range(2):
        r = slice(h * 32, h * 32 + 32)
        i = slice(64 + h * 32, 64 + h * 32 + 32)
        W[r, r] = Lr
        W[r, i] = -Li
        W[i, r] = Li
        W[i, i] = Lr
    return np.ascontiguousarray(W.T)


def _sgn_sum_weights():
    """W so that (W @ sq)[c*64+h*32+m] = sgn(c) * (sq[r_h,m] + sq[i_h,m]),
    sgn(re half) = -1, sgn(im half) = +1. Returns lhsT = W.T."""
    W = np.zeros((128, 128), np.float32)
    I32 = np.eye(32, dtype=np.float32)
    for h in range(2):
        r = slice(h * 32, h * 32 + 32)
        i = slice(64 + h * 32, 64 + h * 32 + 32)
        W[r, r] = -I32
        W[r, i] = -I32
        W[i, r] = I32
        W[i, i] = I32
    return np.ascontiguousarray(W.T)


_PROGRAM_CACHE = {}


def _build_program(nsteps=NSTEPS):
    key = nsteps
    if key in _PROGRAM_CACHE:
        return _PROGRAM_CACHE[key]
    import concourse.bacc as bacc
    import concourse.mybir as mybir
    import concourse.tile as tile

    F32 = mybir.dt.float32
    OP = mybir.AluOpType
    AF = mybir.ActivationFunctionType

    nc = bacc.Bacc(
        "TRN2", target_bir_lowering=False, debug=False, enable_asserts=False
    )
    y0_d = nc.dram_tensor("y0", [128, 128], F32, kind="ExternalInput")
    # stage weights: [0]=dt/2, [1]=dt, [2]=dt/6 ; [3]=sgn-sum ; [4]=identity
    wts_d = nc.dram_tensor("wts", [5, 128, 128], F32, kind="ExternalInput")
    # ACT scales sqrt(s_j*nu) per stage j=0..3 packed host-side into scalars
    # via immediates is not possible (nu is runtime data) -> per-partition
    # scale tensors [128,1] per stage (4 stages).
    scl_d = nc.dram_tensor("scl", [128, 4], F32, kind="ExternalInput")
    traj_d = nc.dram_tensor("traj", [nsteps, 128, 128], F32, kind="ExternalOutput")

    with tile.TileContext(nc) as tc:
        with (
            tc.tile_pool(name="const", bufs=1) as cpool,
            tc.tile_pool(name="state", bufs=1) as spool,
            tc.tile_pool(name="work", bufs=3) as wpool,
            tc.tile_pool(name="yout", bufs=3) as ypool,
            tc.tile_pool(name="psum", bufs=2, space="PSUM") as ppool,
        ):
            wt = []
            for k in range(5):
                w = cpool.tile([128, 128], F32, tag=f"w{k}")
                nc.sync.dma_start(w[:], wts_d.ap()[k])
                wt.append(w)
            scl = cpool.tile([128, 4], F32, tag="scl")
            nc.sync.dma_start(scl[:], scl_d.ap())

            # persistent per-stream state (ping-pong)
            A = [
                [
                    spool.tile(
                        [128, FD], F32, tag=f"A{s}_{p}", name=f"A{s}_{p}"
                    )
                    for p in range(2)
                ]
                for s in range(NSTREAM)
            ]
            for s in range(NSTREAM):
                nc.sync.dma_start(
                    A[s][0][:], y0_d.ap()[:, s * FD : (s + 1) * FD]
                )

            STAGE_W = [0, 0, 1, 2]  # weight index per stage
            for t in range(nsteps):
                cur = t % 2
                nxt = 1 - cur
                for s in range(NSTREAM):
                    Acur = A[s][cur]
                    Ys = [None, None, None]  # Y2, Y3, Y4 tiles
                    Y = Acur
                    M4 = None
                    for j in range(4):
                        wj = wt[STAGE_W[j]]
                        M = ppool.tile([128, FD], F32, tag=f"m{s}")
                        nc.tensor.matmul(
                            M[:], wj[:], Y[:], start=True, stop=False
                        )
                        sq = wpool.tile([128, FD], F32, tag=f"sq{s}")
                        nc.scalar.activation(
                            sq[:], Y[:], AF.Square,
                            bias=0.0, scale=scl[:, j : j + 1],
                        )
                        ab = ppool.tile([128, FD], F32, tag=f"ab{s}")
                        nc.tensor.matmul(
                            ab[:], wt[3][:], sq[:], start=True, stop=True
                        )
                        ysw = wpool.tile([128, FD], F32, tag=f"ysw{s}")
                        nc.gpsimd.tensor_copy(ysw[0:64, :], Y[64:128, :])
                        nc.gpsimd.tensor_copy(ysw[64:128, :], Y[0:64, :])
                        th = wpool.tile([128, FD], F32, tag=f"th{s}")
                        nc.vector.tensor_tensor(th[:], ab[:], ysw[:], OP.mult)
                        nc.tensor.matmul(
                            M[:], wt[4][:], th[:], start=False, stop=True
                        )
                        if j < 3:
                            Yn = ypool.tile([128, FD], F32, tag=f"y{s}_{j}")
                            nc.vector.tensor_tensor(
                                Yn[:], M[:], Acur[:], OP.add
                            )
                            Ys[j] = Yn
                            Y = Yn
                        else:
                            M4 = M
                    # combine: A' = (Y2 + 2*Y3 + Y4 - A)/3 + M4'
                    c1 = wpool.tile([128, FD], F32, tag=f"c1{s}")
                    nc.vector.scalar_tensor_tensor(
                        c1[:], Ys[1][:], 2.0, Ys[0][:], OP.mult, OP.add
                    )
                    c2 = wpool.tile([128, FD], F32, tag=f"c2{s}")
                    nc.gpsimd.tensor_tensor(
                        c2[:], Ys[2][:], Acur[:], OP.subtract
                    )
                    c3 = wpool.tile([128, FD], F32, tag=f"c3{s}")
                    nc.gpsimd.tensor_tensor(c3[:], c1[:], c2[:], OP.add)
                    Anew = A[s][nxt]
                    nc.vector.scalar_tensor_tensor(
                        Anew[:], c3[:], 1.0 / 3.0, M4[:],
                        OP.mult, OP.add,
                    )
                    nc.sync.dma_start(
                        traj_d.ap()[t][:, s * FD : (s + 1) * FD], Anew[:]
                    )
    nc.compile()
    _PROGRAM_CACHE[key] = nc
    return nc


def _prep_inputs(A0_real, A0_imag, omega, kappa, nonlinearity, params):
    Lc, nu = _host_matrices(omega, kappa, nonlinearity, params)
    wts = np.stack(
        [
            _block_weights(Lc, DT / 2.0),
            _block_weights(Lc, DT),
            _block_weights(Lc, DT / 6.0),
            _sgn_sum_weights(),
            np.eye(128, dtype=np.float32),
        ]
    )
    s_coeff = [DT / 2.0, DT / 2.0, DT, DT / 6.0]
    scl = np.zeros((128, 4), np.float32)
    for j in range(4):
        scl[:, j] = np.sqrt(s_coeff[j] * nu)

    # initial state, padded: first 24 modes from input, rest 1.0 + 0j
    Ar = np.ones((BATCH, MODES), np.float32)
    Ai = np.zeros((BATCH, MODES), np.float32)
    Ar[:, :INPUT_MODES] = A0_real
    Ai[:, :INPUT_MODES] = A0_imag

    in_maps = []
    for c in range(NCORES):
        y0 = np.zeros((128, 128), np.float32)
        for s in range(NSTREAM):
            for h in range(2):
                rows = slice(
                    c * B_CORE + s * 2 * FD + h * FD,
                    c * B_CORE + s * 2 * FD + (h + 1) * FD,
                )
                # partitions c*64 + h*32 + m ; free col = s*FD + b
                y0[h * 32 : h * 32 + 32, s * FD : (s + 1) * FD] = Ar[rows].T
                y0[64 + h * 32 : 64 + h * 32 + 32, s * FD : (s + 1) * FD] = (
                    Ai[rows].T
                )
        in_maps.append({"y0": y0, "wts": wts, "scl": scl})
    return in_maps, Ar, Ai


def _assemble(results, Ar, Ai, nsteps=NSTEPS):
    out = np.empty((nsteps + 1, BATCH, MODES), np.complex64)
    out[0] = (Ar + 1j * Ai).astype(np.complex64)
    for c in range(NCORES):
        tr = results[c]["traj"]  # [nsteps, 128, 128]
        # partitions p = cc*64 + h*32 + m ; free col = s*FD + b
        v = tr.reshape(nsteps, 2, 2, 32, NSTREAM, FD)
        # axes: (t, cc, h, m, s, b) -> batch row = c*256 + s*128 + h*64 + b
        arr = (v[:, 0] + 1j * v[:, 1]).astype(np.complex64)  # (t, h, m, s, b)
        arr = arr.transpose(0, 3, 1, 4, 2)  # (t, s, h, b, m)
        out[1:, c * B_CORE : (c + 1) * B_CORE, :] = arr.reshape(
            nsteps, B_CORE, MODES
        )
    return out


def kernel(A0_real, A0_imag, omega, kappa, nonlinearity, params):
    from concourse.bass_utils import run_bass_kernel_spmd

    nc = _build_program(NSTEPS)
    in_maps, Ar, Ai = _prep_inputs(
        np.asarray(A0_real), np.asarray(A0_imag), np.asarray(omega),
        np.asarray(kappa), np.asarray(nonlinearity), np.asarray(params),
    )
    res = run_bass_kernel_spmd(nc, in_maps, core_ids=list(range(NCORES)))
    return _assemble(res.results, Ar, Ai)
